# revision 35
# baseline (speedup 1.0000x reference)
"""Blended-MoE 3-layer MLP (moe_routing) on 8 trn2 NeuronCores.

Math: per layer  z[b,o] = sum_e blend[e,b] * (w[e] @ h[b] + bias[e])[o],
ELU between layers.  Rewritten as a single contraction per layer:

    z[b,o] = sum_{(e,i)} (blend[e,b] * hT[i,b]) * wT[(e,i), o]
           + sum_j blendT[j,b] * bias[j,o]            (bias rows appended to K)

so the expert blend AND the bias become part of the matmul K dimension.
Data-parallel across 8 cores (128 batch rows each); expert weights are
replicated, host-side pre-transposed/padded into SBUF-image layout.
"""

import numpy as np

import bass_rust
import concourse.bass as bass
import concourse.mybir as mybir
import concourse.tile as tile
from concourse.bass_utils import run_bass_kernel_spmd
from concourse.masks import make_identity

# ---- config ----------------------------------------------------------------
N_CORES = 8
B, E = 1024, 8
DIN, D1, D2, D3 = 480, 512, 512, 311
KP = 512          # padded per-expert contraction
NKT = 33          # k-tiles per layer: 8 experts * 4 + 1 aug tile
N_L = (D1, D2, D3)

USE_FP16 = True   # compute dtype for weights/activations (PSUM accum is fp32)

PROFILE = {"trace": False, "tmpdir": None}
LAST_RESULT = [None]

_NC_CACHE = {}
_SPLIT_N = [0]


def _split_multi_waits(nc, max_waits=1):
    """This container's walrus only supports one sync-wait command per
    instruction; spill extras onto same-engine NOPs inserted just before."""
    for f in nc.m.functions:
        for bb in f.blocks:
            insts = bb.instructions
            i = 0
            while i < len(insts):
                inst = insts[i]
                si = inst.sync_info
                if si is not None and len(si.on_wait) > max_waits:
                    waits = list(si.on_wait)
                    extra, keep = waits[:-max_waits], waits[-max_waits:]
                    for w in extra:
                        _SPLIT_N[0] += 1
                        nop = mybir.InstNoOp(
                            name=f"wsplit-{_SPLIT_N[0]}", ins=[], outs=[]
                        )
                        nop.engine = inst.engine
                        nop.sync_info = bass_rust.SyncInfo(
                            on_wait=[w], on_update=[]
                        )
                        insts.insert(i, nop)
                        i += 1
                    inst.sync_info = bass_rust.SyncInfo(
                        on_wait=keep, on_update=list(si.on_update)
                    )
                i += 1


def _patch_minimal_tail():
    """Tile's kernel-tail is drain + 2 full all-engine barriers + sem clear
    (~10us).  Replace with drain + one barrier + range clear."""
    from concourse.vector_clock import ScopedClock

    def _drain_and_barrier(self, tick_clock, wait_clock):
        nc = self.nc
        drain_inst = nc.sync.drain()
        wait_clock.add_sem_waits(
            drain_inst.ins, ScopedClock({None: tick_clock.global_clock})
        )
        nc.all_engine_barrier()
        popped = nc._tile_sem_poison_stack.pop()
        assert popped is self._sem_poison
        assert self.sems is not None
        nc.clear_and_free_semaphores(list(self.sems.allocated().values()))
        # original ends with a second all_engine_barrier; the gpsimd range
        # clear is the last thing this engine does and the next NEFF
        # execution starts only after every engine ended, so skip it.

    tile.TileContext._drain_and_barrier = _drain_and_barrier


import os as _os

if _os.environ.get("KERNEL_MIN_TAIL", "0") == "1":
    _patch_minimal_tail()


def _build_nc(dt_my, wbufs):
    f32 = mybir.dt.float32
    nc = bass.Bass()

    w_d = [
        nc.dram_tensor(f"w{l}s", [128, NKT * n], dt_my, kind="ExternalInput")
        for l, n in enumerate(N_L)
    ]
    # misc = [xT(512) | aug(128) | w0aug(512) | w1aug(512) | w2aug(311)]
    MISC_COLS = 512 + 128 + D1 + D2 + D3
    misc_d = nc.dram_tensor("misc", [128, MISC_COLS], dt_my, kind="ExternalInput")
    bb_d = nc.dram_tensor("bb", [128, E * 512], dt_my, kind="ExternalInput")
    out_d = nc.dram_tensor("out", [128, D3], f32, kind="ExternalOutput")

    with tile.TileContext(nc) as tc:
        with (
            tc.tile_pool(name="const", bufs=1) as const,
            tc.tile_pool(name="w", bufs=wbufs) as wpool,
            tc.tile_pool(name="acts", bufs=2) as acts,
            tc.tile_pool(name="tmp", bufs=2) as tmp,
            tc.tile_pool(name="zp", bufs=2, space="PSUM") as zp,
            tc.tile_pool(name="tp", bufs=2, space="PSUM") as tp,
        ):
            # weight groups per layer: 8 k-tiles per ~1MB DMA (big per-
            # partition runs keep the 16 DMA engines at full rate; several
            # concurrent DMAs keep all DGE queues active)
            GSIZES = (8, 8, 8, 8)
            wg = [[] for _ in N_L]      # [(tile, start_kt, n_kt)]
            waug = [None] * 3

            fp16 = dt_my == mybir.dt.float16
            WBUFS = {8: 12 if fp16 else 4}

            def _wdma(l, g):
                n = N_L[l]
                sz = GSIZES[g]
                start = sum(GSIZES[:g])
                t = wpool.tile(
                    [128, sz * n], dt_my, tag=f"w{sz}", bufs=WBUFS[sz]
                )
                nc.sync.dma_start(
                    t[:], w_d[l][:, start * n : (start + sz) * n]
                )
                wg[l].append((t, start, sz))

            # one DMA for all the small inputs (small-packet DMAs clog a DGE
            # queue for ~10us each; merged they land in one shot)
            misc_sb = const.tile([128, MISC_COLS], dt_my)
            nc.sync.dma_start(misc_sb[:], misc_d[:])
            xt_sb = misc_sb[:, 0:512]
            aug_sb = misc_sb[:, 512:640]
            off = 640
            for l, n in enumerate(N_L):
                waug[l] = misc_sb[:, off : off + n]
                off += n

            bb_sb = const.tile([128, E * 512], dt_my)
            nc.sync.dma_start(bb_sb[:], bb_d[:])
            ident = const.tile([128, 128], dt_my)
            make_identity(nc, ident[:])
            for l in range(3):
                for g in range(4):
                    _wdma(l, g)

            src = xt_sb  # transposed activations for layer 0 (4 k-tiles wide)
            for l, n in enumerate(N_L):
                # expand: he[:, e*512:(e+1)*512] = src * blend_bcast[e]
                he = acts.tile([128, 32 * 128], dt_my, tag="he")
                for e in range(E):
                    nc.vector.tensor_tensor(
                        he[:, e * 512 : (e + 1) * 512],
                        src[:],
                        bb_sb[:, e * 512 : (e + 1) * 512],
                        mybir.AluOpType.mult,
                    )

                # contraction: aug tile first (ready immediately), then 32
                # expanded tiles
                z = zp.tile([128, n], f32, tag="z")
                nc.tensor.matmul(
                    z[:], aug_sb[:], waug[l][:, :n], start=True, stop=False
                )
                for wt, start, sz in wg[l]:
                    for loc in range(sz):
                        t = start + loc
                        nc.tensor.matmul(
                            z[:],
                            he[:, t * 128 : (t + 1) * 128],
                            wt[:, loc * n : (loc + 1) * n],
                            start=False,
                            stop=(t == 31),
                        )

                if l == 2:
                    out_sb = tmp.tile([128, D3], f32, tag="osb")
                    nc.scalar.copy(out_sb[:], z[:])
                    nc.sync.dma_start(out_d[:], out_sb[:])
                    break

                # ELU(z) = (max(z,0) - 1) + exp(min(z,0))
                m = tmp.tile([128, n], f32, tag="m")
                nc.vector.tensor_scalar(
                    m[:], z[:], 0.0, None, mybir.AluOpType.min
                )
                p = tmp.tile([128, n], f32, tag="p")
                nc.vector.tensor_scalar(
                    p[:], z[:], 0.0, -1.0,
                    mybir.AluOpType.max, mybir.AluOpType.add,
                )
                ex = tmp.tile([128, n], f32, tag="ex")
                nc.scalar.activation(
                    ex[:], m[:], mybir.ActivationFunctionType.Exp
                )
                h = tmp.tile([128, n], dt_my, tag="h")
                nc.vector.tensor_tensor(
                    h[:], p[:], ex[:], mybir.AluOpType.add
                )

                # transpose h (128, 512) -> hT as 4 stacked 128x128 tiles
                tps = tp.tile([128, 4 * 128], dt_my, tag="tps")
                for it in range(4):
                    nc.tensor.transpose(
                        tps[:, it * 128 : (it + 1) * 128],
                        h[:, it * 128 : (it + 1) * 128],
                        ident[:],
                    )
                hT = tmp.tile([128, 4 * 128], dt_my, tag="hT")
                nc.scalar.copy(hT[:], tps[:])
                src = hT

    _split_multi_waits(nc)
    return nc


# ---- host-side packing -----------------------------------------------------


def _wimg(w, bvec, np_dt):
    """(E, dout, din) weights + (E, dout) bias -> (128, 33*dout) SBUF image."""
    e_, dout, din = w.shape
    img = np.zeros((NKT * 128, dout), np.float32)
    for e in range(e_):
        img[e * KP : e * KP + din] = w[e].T
    img[e_ * KP : e_ * KP + e_] = bvec
    return np.ascontiguousarray(
        img.reshape(NKT, 128, dout).transpose(1, 0, 2).reshape(128, NKT * dout)
    ).astype(np_dt)


def kernel(x, weight_blend, w0, b0, w1, b1, w2, b2):
    np_dt = np.float16 if USE_FP16 else np.float32
    dt_my = mybir.dt.float16 if USE_FP16 else mybir.dt.float32

    key = (USE_FP16,)
    if key not in _NC_CACHE:
        _NC_CACHE[key] = _build_nc(dt_my, wbufs=12 if USE_FP16 else 6)
    nc = _NC_CACHE[key]

    x = np.asarray(x, np.float32)
    weight_blend = np.asarray(weight_blend, np.float32)
    wimgs = {
        "w0s": _wimg(np.asarray(w0), np.asarray(b0), np_dt),
        "w1s": _wimg(np.asarray(w1), np.asarray(b1), np_dt),
        "w2s": _wimg(np.asarray(w2), np.asarray(b2), np_dt),
    }

    bc = B // N_CORES
    in_maps = []
    for c in range(N_CORES):
        sl = slice(c * bc, (c + 1) * bc)
        xT = np.zeros((4 * 128, bc), np.float32)
        xT[:DIN] = x[sl].T
        xt_img = xT.reshape(4, 128, bc).transpose(1, 0, 2).reshape(128, 4 * bc)
        bl = weight_blend[:, sl]  # (8, 128)
        # bb[p, e*512 + it*128 + b] = bl[e, b]  (it in 0..3)
        bb_img = np.broadcast_to(
            bl[:, None, None, :], (E, 4, 128, bc)
        ).transpose(2, 0, 1, 3).reshape(128, E * 4 * bc)
        aug_img = np.zeros((128, bc), np.float32)
        aug_img[:E] = bl
        misc = np.concatenate(
            [
                np.ascontiguousarray(xt_img).astype(np_dt),
                aug_img.astype(np_dt),
                wimgs["w0s"][:, 32 * D1 : 33 * D1],
                wimgs["w1s"][:, 32 * D2 : 33 * D2],
                wimgs["w2s"][:, 32 * D3 : 33 * D3],
            ],
            axis=1,
        )
        in_maps.append(
            {
                **wimgs,
                "misc": np.ascontiguousarray(misc),
                "bb": np.ascontiguousarray(bb_img).astype(np_dt),
            }
        )

    res = run_bass_kernel_spmd(
        nc,
        in_maps,
        core_ids=list(range(N_CORES)),
        trace=PROFILE["trace"],
        tmpdir=PROFILE["tmpdir"],
    )
    LAST_RESULT[0] = res
    return np.concatenate(
        [res.results[c]["out"] for c in range(N_CORES)], axis=0
    )


# revision 37
# speedup vs baseline: 1.0571x; 1.0571x over previous
"""Blended-MoE 3-layer MLP (moe_routing) on 8 trn2 NeuronCores.

Math: per layer  z[b,o] = sum_e blend[e,b] * (w[e] @ h[b] + bias[e])[o],
ELU between layers.  Rewritten as a single contraction per layer:

    z[b,o] = sum_{(e,i)} (blend[e,b] * hT[i,b]) * wT[(e,i), o]
           + sum_j blendT[j,b] * bias[j,o]            (bias rows appended to K)

so the expert blend AND the bias become part of the matmul K dimension.
Data-parallel across 8 cores (128 batch rows each); expert weights are
replicated, host-side pre-transposed/padded into SBUF-image layout.
"""

import numpy as np

import bass_rust
import concourse.bass as bass
import concourse.mybir as mybir
import concourse.tile as tile
from concourse.bass_utils import run_bass_kernel_spmd
from concourse.masks import make_identity

# ---- config ----------------------------------------------------------------
N_CORES = 8
B, E = 1024, 8
DIN, D1, D2, D3 = 480, 512, 512, 311
KP = 512          # padded per-expert contraction
NKT = 33          # k-tiles per layer: 8 experts * 4 + 1 aug tile
N_L = (D1, D2, D3)

USE_FP16 = True   # compute dtype for weights/activations (PSUM accum is fp32)

PROFILE = {"trace": False, "tmpdir": None}
LAST_RESULT = [None]

_NC_CACHE = {}
_SPLIT_N = [0]


def _split_multi_waits(nc, max_waits=1):
    """This container's walrus only supports one sync-wait command per
    instruction; spill extras onto same-engine NOPs inserted just before."""
    for f in nc.m.functions:
        for bb in f.blocks:
            insts = bb.instructions
            i = 0
            while i < len(insts):
                inst = insts[i]
                si = inst.sync_info
                if si is not None and len(si.on_wait) > max_waits:
                    waits = list(si.on_wait)
                    extra, keep = waits[:-max_waits], waits[-max_waits:]
                    for w in extra:
                        _SPLIT_N[0] += 1
                        nop = mybir.InstNoOp(
                            name=f"wsplit-{_SPLIT_N[0]}", ins=[], outs=[]
                        )
                        nop.engine = inst.engine
                        nop.sync_info = bass_rust.SyncInfo(
                            on_wait=[w], on_update=[]
                        )
                        insts.insert(i, nop)
                        i += 1
                    inst.sync_info = bass_rust.SyncInfo(
                        on_wait=keep, on_update=list(si.on_update)
                    )
                i += 1


def _patch_minimal_tail():
    """Tile's kernel-tail is drain + 2 full all-engine barriers + sem clear
    (~10us).  Replace with drain + one barrier + range clear."""
    from concourse.vector_clock import ScopedClock

    def _drain_and_barrier(self, tick_clock, wait_clock):
        nc = self.nc
        drain_inst = nc.sync.drain()
        wait_clock.add_sem_waits(
            drain_inst.ins, ScopedClock({None: tick_clock.global_clock})
        )
        nc.all_engine_barrier()
        popped = nc._tile_sem_poison_stack.pop()
        assert popped is self._sem_poison
        assert self.sems is not None
        nc.clear_and_free_semaphores(list(self.sems.allocated().values()))
        # original ends with a second all_engine_barrier; the gpsimd range
        # clear is the last thing this engine does and the next NEFF
        # execution starts only after every engine ended, so skip it.

    tile.TileContext._drain_and_barrier = _drain_and_barrier


import os as _os

if _os.environ.get("KERNEL_MIN_TAIL", "0") == "1":
    _patch_minimal_tail()


def _build_nc(dt_my, wbufs):
    f32 = mybir.dt.float32
    nc = bass.Bass()

    w_d = [
        nc.dram_tensor(f"w{l}s", [128, NKT * n], dt_my, kind="ExternalInput")
        for l, n in enumerate(N_L)
    ]
    # misc = [xT(512) | aug(128) | w0aug(512) | w1aug(512) | w2aug(311)]
    MISC_COLS = 512 + 128 + D1 + D2 + D3
    misc_d = nc.dram_tensor("misc", [128, MISC_COLS], dt_my, kind="ExternalInput")
    bb_d = nc.dram_tensor("bb", [128, E * 512], dt_my, kind="ExternalInput")
    out_d = nc.dram_tensor("out", [128, D3], f32, kind="ExternalOutput")

    with tile.TileContext(nc) as tc:
        with (
            tc.tile_pool(name="const", bufs=1) as const,
            tc.tile_pool(name="w", bufs=wbufs) as wpool,
            tc.tile_pool(name="acts", bufs=2) as acts,
            tc.tile_pool(name="tmp", bufs=2) as tmp,
            tc.tile_pool(name="zp", bufs=2, space="PSUM") as zp,
            tc.tile_pool(name="tp", bufs=2, space="PSUM") as tp,
        ):
            # weight groups per layer: 8 k-tiles per ~1MB DMA (big per-
            # partition runs keep the 16 DMA engines at full rate; several
            # concurrent DMAs keep all DGE queues active)
            GSIZES = (8, 8, 8, 8)
            wg = [[] for _ in N_L]      # [(tile, start_kt, n_kt)]
            waug = [None] * 3

            fp16 = dt_my == mybir.dt.float16
            WBUFS = {8: 12 if fp16 else 4}

            def _wdma(l, g):
                n = N_L[l]
                sz = GSIZES[g]
                start = sum(GSIZES[:g])
                t = wpool.tile(
                    [128, sz * n], dt_my, tag=f"w{sz}", bufs=WBUFS[sz]
                )
                nc.sync.dma_start(
                    t[:], w_d[l][:, start * n : (start + sz) * n]
                )
                wg[l].append((t, start, sz))

            # one DMA for all the small inputs (small-packet DMAs clog a DGE
            # queue for ~10us each; merged they land in one shot)
            misc_sb = const.tile([128, MISC_COLS], dt_my)
            nc.sync.dma_start(misc_sb[:], misc_d[:])
            xt_sb = misc_sb[:, 0:512]
            aug_sb = misc_sb[:, 512:640]
            off = 640
            for l, n in enumerate(N_L):
                waug[l] = misc_sb[:, off : off + n]
                off += n

            bb_sb = const.tile([128, E * 512], dt_my)
            nc.sync.dma_start(bb_sb[:], bb_d[:])
            ident = const.tile([128, 128], dt_my)
            make_identity(nc, ident[:])
            for l in range(3):
                for g in range(4):
                    _wdma(l, g)

            src = xt_sb  # transposed activations for layer 0 (4 k-tiles wide)
            for l, n in enumerate(N_L):
                # expand: he[:, e*512:(e+1)*512] = src * blend_bcast[e]
                he = acts.tile([128, 32 * 128], dt_my, tag="he")
                for e in range(E):
                    nc.vector.tensor_tensor(
                        he[:, e * 512 : (e + 1) * 512],
                        src[:],
                        bb_sb[:, e * 512 : (e + 1) * 512],
                        mybir.AluOpType.mult,
                    )

                # contraction: aug tile first (ready immediately), then 32
                # expanded tiles
                z = zp.tile([128, n], f32, tag="z")
                nc.tensor.matmul(
                    z[:], aug_sb[:], waug[l][:, :n], start=True, stop=False
                )
                for wt, start, sz in wg[l]:
                    for loc in range(sz):
                        t = start + loc
                        nc.tensor.matmul(
                            z[:],
                            he[:, t * 128 : (t + 1) * 128],
                            wt[:, loc * n : (loc + 1) * n],
                            start=False,
                            stop=(t == 31),
                        )

                if l == 2:
                    out_sb = tmp.tile([128, D3], f32, tag="osb")
                    nc.scalar.copy(out_sb[:], z[:])
                    nc.sync.dma_start(out_d[:], out_sb[:])
                    break

                # ELU(z) = (max(z,0) - 1) + exp(min(z,0))
                m = tmp.tile([128, n], f32, tag="m")
                nc.vector.tensor_scalar(
                    m[:], z[:], 0.0, None, mybir.AluOpType.min
                )
                p = tmp.tile([128, n], f32, tag="p")
                nc.vector.tensor_scalar(
                    p[:], z[:], 0.0, -1.0,
                    mybir.AluOpType.max, mybir.AluOpType.add,
                )
                ex = tmp.tile([128, n], f32, tag="ex")
                nc.scalar.activation(
                    ex[:], m[:], mybir.ActivationFunctionType.Exp
                )
                h = tmp.tile([128, n], dt_my, tag="h")
                nc.vector.tensor_tensor(
                    h[:], p[:], ex[:], mybir.AluOpType.add
                )

                # transpose h (128, 512) -> hT as 4 stacked 128x128 tiles
                tps = tp.tile([128, 4 * 128], dt_my, tag="tps")
                for it in range(4):
                    nc.tensor.transpose(
                        tps[:, it * 128 : (it + 1) * 128],
                        h[:, it * 128 : (it + 1) * 128],
                        ident[:],
                    )
                hT = tmp.tile([128, 4 * 128], dt_my, tag="hT")
                nc.scalar.copy(hT[:], tps[:])
                src = hT

    _split_multi_waits(nc)
    return nc


# ---- host-side packing -----------------------------------------------------


def _wimg(w, bvec, np_dt):
    """(E, dout, din) weights + (E, dout) bias -> (128, 33*dout) SBUF image."""
    e_, dout, din = w.shape
    img = np.zeros((NKT * 128, dout), np.float32)
    for e in range(e_):
        img[e * KP : e * KP + din] = w[e].T
    img[e_ * KP : e_ * KP + e_] = bvec
    return np.ascontiguousarray(
        img.reshape(NKT, 128, dout).transpose(1, 0, 2).reshape(128, NKT * dout)
    ).astype(np_dt)


def kernel(x, weight_blend, w0, b0, w1, b1, w2, b2):
    np_dt = np.float16 if USE_FP16 else np.float32
    dt_my = mybir.dt.float16 if USE_FP16 else mybir.dt.float32

    key = (USE_FP16,)
    if key not in _NC_CACHE:
        _NC_CACHE[key] = _build_nc(dt_my, wbufs=12 if USE_FP16 else 6)
    nc = _NC_CACHE[key]

    x = np.asarray(x, np.float32)
    weight_blend = np.asarray(weight_blend, np.float32)
    wimgs = {
        "w0s": _wimg(np.asarray(w0), np.asarray(b0), np_dt),
        "w1s": _wimg(np.asarray(w1), np.asarray(b1), np_dt),
        "w2s": _wimg(np.asarray(w2), np.asarray(b2), np_dt),
    }

    bc = B // N_CORES
    in_maps = []
    for c in range(N_CORES):
        sl = slice(c * bc, (c + 1) * bc)
        xT = np.zeros((4 * 128, bc), np.float32)
        xT[:DIN] = x[sl].T
        xt_img = xT.reshape(4, 128, bc).transpose(1, 0, 2).reshape(128, 4 * bc)
        bl = weight_blend[:, sl]  # (8, 128)
        # bb[p, e*512 + it*128 + b] = bl[e, b]  (it in 0..3)
        bb_img = np.broadcast_to(
            bl[:, None, None, :], (E, 4, 128, bc)
        ).transpose(2, 0, 1, 3).reshape(128, E * 4 * bc)
        aug_img = np.zeros((128, bc), np.float32)
        aug_img[:E] = bl
        misc = np.concatenate(
            [
                np.ascontiguousarray(xt_img).astype(np_dt),
                aug_img.astype(np_dt),
                wimgs["w0s"][:, 32 * D1 : 33 * D1],
                wimgs["w1s"][:, 32 * D2 : 33 * D2],
                wimgs["w2s"][:, 32 * D3 : 33 * D3],
            ],
            axis=1,
        )
        in_maps.append(
            {
                **wimgs,
                "misc": np.ascontiguousarray(misc),
                "bb": np.ascontiguousarray(bb_img).astype(np_dt),
            }
        )

    res = run_bass_kernel_spmd(
        nc,
        in_maps,
        core_ids=list(range(N_CORES)),
        trace=PROFILE["trace"],
        tmpdir=PROFILE["tmpdir"],
    )
    LAST_RESULT[0] = res
    return np.concatenate(
        [res.results[c]["out"] for c in range(N_CORES)], axis=0
    )
